# revision 1
# baseline (speedup 1.0000x reference)
"""Causal self-attention (B=4, T=2048, C=1024, H=16, D=64) on 8 TRN2 NeuronCores.

Sharding: batch x head-group. Core c handles batch b = c//2 and heads
hg*8..hg*8+8 where hg = c%2 (data parallel on batch, tensor parallel on heads;
w_qkv column-sharded, w_out row-sharded). Each core is fully independent; the
host sums the two per-batch partial outputs and adds the bias terms.

All matmuls run in float32r (fp32 with the mantissa RNE-rounded to 11 explicit
bits; full 1 column/cycle PE rate for moving dims >= 256). DRAM inputs that
feed matmuls are pre-rounded to the fp32r bit format on the host and declared
float32r end to end; every on-chip matmul operand is produced by an
instruction whose output dtype is float32r, which performs the rounding.

Per-core kernel:
  Phase B: QKV projection. x arrives pre-transposed (xT [C,T]) so
    Q^T/K^T [qk-col, t] come out of the PE directly (lhsT=w1, rhs=xT), and
    V [t, v-col] comes from the swapped orientation (lhsT=xT, rhs=w1v).
    V is stored packed per t-tile as [128, 8*(64+1)] with a ones column per
    head, which makes the PV matmul also emit the softmax denominator.
  Phase C: attention per head-pair. S^T [j,t_q] = K^T.T @ Q^T (row-packed
    pairs of K=64 matmuls), exp on ScalarE straight out of PSUM into P^T in
    SBUF (no max subtraction: scores are O(1) by construction), causal
    masking via affine_select on diagonal blocks + narrowed PV ranges.
    O^T[d, t_q] (+ sum row) = [V|1].T @ P^T accumulated in PSUM. Normalize
    with reciprocal + K=1 ones-matmul broadcast + tensor_mul.
  Phase D: out projection y[t, c] = O^T.T @ w2, K=128 over pair-stacked O.
"""

import numpy as np

import concourse.bass as bass
import concourse.bacc as bacc
import concourse.mybir as mybir
from concourse.tile import TileContext

# ---- problem constants (hardcoded per contract) ----
B, T, C = 4, 2048, 1024
H_GLOBAL, D = 16, 64
HL = 8                      # local heads per core
N_CORES = 8
P = 128
KT_C = C // P               # 8 contraction tiles over C
NT = T // P                 # 16 t-tiles
IB = 512                    # query block (i-chunk)
NIC = T // IB               # 4 i-chunks
G = 2                       # j-tiles per exp group
XB = 256                    # x stream chunk width (t columns)
M1 = 3 * HL * D             # 1536 local qkv cols
F32 = mybir.dt.float32
F32R = mybir.dt.float32r
SCALE = 1.0 / np.sqrt(D).astype(np.float32)


def round_f32r(a):
    """Host-side RNE rounding of fp32 to the fp32r bit format (11-bit
    explicit mantissa). Matches libwalrus fp32_to_fp32r exactly."""
    u = np.ascontiguousarray(a, dtype=np.float32).view(np.uint32).astype(np.uint64)
    low = u & 0xFFF
    keep = u >> 12
    roundup = (low > 0x800) | ((low == 0x800) & ((keep & 1) == 1))
    return ((keep + roundup.astype(np.uint64)) << 12).astype(np.uint32).view(
        np.float32)


def build_nc(repeat=1, phases="BCD"):
    from concourse import library_config
    nc = bacc.Bacc("TRN2", target_bir_lowering=False)
    EXPF = mybir.ActivationFunctionType.Exp

    xT = nc.dram_tensor("xT", [C, T], F32R, kind="ExternalInput").ap()
    w1 = nc.dram_tensor("w1", [C, M1], F32R, kind="ExternalInput").ap()
    b1 = nc.dram_tensor("b1", [M1], F32, kind="ExternalInput").ap()
    w2 = nc.dram_tensor("w2", [HL * D, C], F32R, kind="ExternalInput").ap()
    y = nc.dram_tensor("y", [T, C], F32, kind="ExternalOutput").ap()

    with TileContext(nc) as tc:
      for _rep in range(repeat):
        with tc.tile_pool(name="persist", bufs=1) as persist:
            ones_f32 = persist.tile([P, D], F32, tag="ones_f32", name="ones_f32")
            nc.vector.memset(ones_f32[:, :], 1.0)
            ones_col = persist.tile([P, D], F32R, tag="ones", name="ones_col")
            nc.vector.tensor_copy(out=ones_col[:, :], in_=ones_f32[:, :])
            b1_sb = persist.tile([P, 12], F32, tag="b1", name="b1_sb")
            nc.sync.dma_start(out=b1_sb[:, :], in_=b1.rearrange("(m p) -> p m", p=P))

            # persistent activations
            QT = [persist.tile([P, T], F32R, tag=f"QT{pr}", name=f"QT{pr}")
                  for pr in range(HL // 2)]
            KT = [persist.tile([P, T], F32R, tag=f"KT{pr}", name=f"KT{pr}")
                  for pr in range(HL // 2)]
            V = [persist.tile([P, HL * (D + 1)], F32R, tag=f"V{tt}", name=f"V{tt}")
                 for tt in range(NT)]
            OP = [persist.tile([P, T], F32R, tag=f"OP{pr}", name=f"OP{pr}")
                  for pr in range(HL // 2)]

            # ---------------- Phase B: QKV projection ----------------
            with tc.tile_pool(name="w1p", bufs=1) as w1p, \
                 tc.tile_pool(name="xs", bufs=2) as xs, \
                 tc.tile_pool(name="ppsum", bufs=4, space="PSUM") as ppsum:
                w1_sb = w1p.tile([P, KT_C, M1], F32R, name="w1_sb")
                nc.sync.dma_start(out=w1_sb[:, :, :],
                                  in_=w1.rearrange("(k p) m -> p k m", p=P))
                for tt in range(NT):
                    nc.vector.tensor_copy(
                        out=V[tt].rearrange("p (h x) -> p h x", x=D + 1)[:, :, D:D + 1],
                        in_=ones_f32.rearrange("p (h o) -> p h o", o=1)[:, 0:HL, :])

                xT_r = xT.rearrange("(k p) t -> p k t", p=P)
                for nch in range(T // XB):
                    xc = xs.tile([P, KT_C, XB], F32R, tag="xc", name="xc")
                    nc.sync.dma_start(out=xc[:, :, :],
                                      in_=xT_r[:, :, nch * XB:(nch + 1) * XB])
                    # V for the t-tiles of this chunk: lhsT = xT, rhs = w1v
                    for ttl in range(XB // P):
                        tt = nch * (XB // P) + ttl
                        vp = ppsum.tile([P, HL * D], F32, tag="pp", name="vp")
                        for k in range(KT_C):
                            nc.tensor.matmul(
                                vp[:, :],
                                lhsT=xc[:, k, ttl * P:(ttl + 1) * P],
                                rhs=w1_sb[:, k, 2 * HL * D:3 * HL * D],
                                start=(k == 0), stop=(k == KT_C - 1))
                        nc.vector.tensor_copy(
                            out=V[tt].rearrange("p (h x) -> p h x", x=D + 1)[:, :, 0:D],
                            in_=vp.rearrange("p (h x) -> p h x", x=D))
                    # Q^T / K^T: lhsT = w1 column block, rhs = xT chunk
                    for pr in range(HL // 2):
                        qp = ppsum.tile([P, XB], F32, tag="pp", name="qp")
                        for k in range(KT_C):
                            nc.tensor.matmul(
                                qp[:, :],
                                lhsT=w1_sb[:, k, pr * P:(pr + 1) * P],
                                rhs=xc[:, k, :],
                                start=(k == 0), stop=(k == KT_C - 1))
                        nc.vector.tensor_scalar_add(
                            QT[pr][:, nch * XB:(nch + 1) * XB], qp[:, :],
                            b1_sb[:, pr:pr + 1])
                        kp = ppsum.tile([P, XB], F32, tag="pp", name="kp")
                        for k in range(KT_C):
                            nc.tensor.matmul(
                                kp[:, :],
                                lhsT=w1_sb[:, k, HL * D + pr * P:HL * D + (pr + 1) * P],
                                rhs=xc[:, k, :],
                                start=(k == 0), stop=(k == KT_C - 1))
                        nc.vector.tensor_scalar_add(
                            KT[pr][:, nch * XB:(nch + 1) * XB], kp[:, :],
                            b1_sb[:, 4 + pr:4 + pr + 1])

            # ---------------- Phase C: attention ----------------
            if "C" not in phases:
                pass
            else:
             with tc.tile_pool(name="spsum", bufs=2, space="PSUM") as spool, \
                 tc.tile_pool(name="opsum", bufs=2, space="PSUM") as opool, \
                 tc.tile_pool(name="bcpsum", bufs=2, space="PSUM") as bcp, \
                 tc.tile_pool(name="ptp", bufs=3) as ptp, \
                 tc.tile_pool(name="recp", bufs=2) as recp, \
                 tc.tile_pool(name="bncp", bufs=2) as bncp:
                for pr in range(HL // 2):
                    for ic in range(NIC):
                        njt = (ic + 1) * (IB // P)
                        ops = [opool.tile([D + 1, IB], F32, tag="op", name=f"o{h2}")
                               for h2 in range(2)]
                        ngr = (njt + G - 1) // G
                        for g in range(ngr):
                            jts = list(range(g * G, min((g + 1) * G, njt)))
                            for h2 in range(2):
                                hs = h2 * D
                                sp = spool.tile([P, G * IB], F32, tag="sp", name="sp")
                                for jl, jt in enumerate(jts):
                                    # full width: garbage j>i columns are
                                    # masked / excluded downstream
                                    nc.tensor.matmul(
                                        sp[:, jl * IB:(jl + 1) * IB],
                                        lhsT=KT[pr][hs:hs + D, jt * P:(jt + 1) * P],
                                        rhs=QT[pr][hs:hs + D,
                                                   ic * IB:(ic + 1) * IB],
                                        start=True, stop=True)
                                pt = ptp.tile([P, G * IB], F32R, tag="pt", name="pt")
                                nc.scalar.activation(pt[:, 0:len(jts) * IB],
                                                     sp[:, 0:len(jts) * IB],
                                                     EXPF, scale=float(SCALE))
                                for jl, jt in enumerate(jts):
                                    rel = jt * P - ic * IB
                                    if 0 <= rel:  # diagonal block: zero j > i
                                        nc.gpsimd.affine_select(
                                            out=pt[:, jl * IB + rel:jl * IB + rel + P],
                                            in_=pt[:, jl * IB + rel:jl * IB + rel + P],
                                            pattern=[[1, P]],
                                            compare_op=mybir.AluOpType.is_ge,
                                            fill=0.0,
                                            base=0,
                                            channel_multiplier=-1)
                                h = pr * 2 + h2
                                for jl, jt in enumerate(jts):
                                    rel = max(0, jt * P - ic * IB)
                                    nc.tensor.matmul(
                                        ops[h2][:, rel:IB],
                                        lhsT=V[jt][:, h * (D + 1):(h + 1) * (D + 1)],
                                        rhs=pt[:, jl * IB + rel:(jl + 1) * IB],
                                        start=(jt == 0), stop=(jt == njt - 1))
                        # normalize: O^T[d, i] / sum_row. Evacuate raw O^T +
                        # sums to SBUF, reciprocal the sums row, round it to
                        # f32r, broadcast across partitions with a K=1
                        # ones-matmul, multiply (rounds to f32r on write).
                        for h2 in range(2):
                            orw = recp.tile([D + 1, IB], F32, tag="orw", name="orw")
                            nc.any.tensor_copy(out=orw[:, :], in_=ops[h2][:, :])
                            nc.vector.reciprocal(orw[D:D + 1, :], orw[D:D + 1, :])
                            rcr = recp.tile([D + 1, IB], F32R, tag="rcr", name="rcr")
                            nc.any.tensor_copy(out=rcr[D:D + 1, :],
                                               in_=orw[D:D + 1, :])
                            bc = bcp.tile([D, IB], F32, tag="bc", name="bc")
                            nc.tensor.matmul(bc[:, :],
                                             lhsT=ones_col[D:D + 1, 0:D],
                                             rhs=rcr[D:D + 1, :],
                                             start=True, stop=True)
                            if h2 == 0:
                                nc.vector.tensor_mul(
                                    out=OP[pr][0:D, ic * IB:(ic + 1) * IB],
                                    in0=orw[0:D, :], in1=bc[:, :])
                            else:
                                bn = bncp.tile([D, IB], F32R, tag="bn", name="bn")
                                nc.vector.tensor_mul(out=bn[:, :],
                                                     in0=orw[0:D, :], in1=bc[:, :])
                                nc.sync.dma_start(
                                    out=OP[pr][D:2 * D, ic * IB:(ic + 1) * IB],
                                    in_=bn[:, :])

            # ---------------- Phase D: output projection ----------------
            if "D" not in phases:
                pass
            else:
             with tc.tile_pool(name="w2p", bufs=1) as w2p, \
                 tc.tile_pool(name="yout", bufs=3) as yout, \
                 tc.tile_pool(name="ypsum", bufs=4, space="PSUM") as ypsum:
                w2_sb = w2p.tile([P, HL * D // P, C], F32R, name="w2_sb")
                nc.sync.dma_start(out=w2_sb[:, :, :],
                                  in_=w2.rearrange("(k p) c -> p k c", p=P))
                for tt in range(NT):
                    for cc in range(C // 512):
                        yp = ypsum.tile([P, 512], F32, tag="yp", name="yp")
                        for pr in range(HL // 2):
                            nc.tensor.matmul(
                                yp[:, :],
                                lhsT=OP[pr][:, tt * P:(tt + 1) * P],
                                rhs=w2_sb[:, pr, cc * 512:(cc + 1) * 512],
                                start=(pr == 0), stop=(pr == HL // 2 - 1))
                        ysb = yout.tile([P, 512], F32, tag="ysb", name="ysb")
                        nc.vector.tensor_copy(out=ysb[:, :], in_=yp[:, :])
                        nc.sync.dma_start(
                            out=y[tt * P:(tt + 1) * P, cc * 512:(cc + 1) * 512],
                            in_=ysb[:, :])
    nc.compile()
    return nc


_NC_CACHE = None


def _get_nc():
    global _NC_CACHE
    if _NC_CACHE is None:
        _NC_CACHE = build_nc()
    return _NC_CACHE


def shard_inputs(x, w_qkv, b_qkv, w_out):
    """Build the 8 per-core input maps (matmul inputs pre-rounded to f32r)."""
    x = np.asarray(x, dtype=np.float32)
    w_qkv = round_f32r(np.asarray(w_qkv, dtype=np.float32))
    b_qkv = np.asarray(b_qkv, dtype=np.float32)
    w_out = round_f32r(np.asarray(w_out, dtype=np.float32))
    in_maps = []
    for core in range(N_CORES):
        b, hg = core // 2, core % 2
        cs = hg * HL * D              # 512-wide contiguous head-group slice
        w1 = np.ascontiguousarray(np.concatenate(
            [w_qkv[:, cs:cs + HL * D],
             w_qkv[:, C + cs:C + cs + HL * D],
             w_qkv[:, 2 * C + cs:2 * C + cs + HL * D]], axis=1))
        b1 = np.ascontiguousarray(np.concatenate(
            [b_qkv[cs:cs + HL * D],
             b_qkv[C + cs:C + cs + HL * D],
             b_qkv[2 * C + cs:2 * C + cs + HL * D]]))
        in_maps.append({
            "xT": round_f32r(x[b].T),
            "w1": w1,
            "b1": b1,
            "w2": np.ascontiguousarray(w_out[cs:cs + HL * D, :]),
        })
    return in_maps


def combine_outputs(results, b_qkv, w_out, b_out):
    """Sum per-batch partials from the two head-group cores + bias terms."""
    bias_vec = (np.asarray(b_qkv[2 * C:3 * C], dtype=np.float32) @
                np.asarray(w_out, dtype=np.float32) +
                np.asarray(b_out, dtype=np.float32))
    y = np.empty((B, T, C), dtype=np.float32)
    for b in range(B):
        y[b] = results[2 * b]["y"] + results[2 * b + 1]["y"] + bias_vec
    return y


def kernel(x, w_qkv, b_qkv, w_out, b_out, *, trace=False, _sink=None):
    from concourse.bass_utils import run_bass_kernel_spmd
    nc = _get_nc()
    in_maps = shard_inputs(x, w_qkv, b_qkv, w_out)
    res = run_bass_kernel_spmd(nc, in_maps, core_ids=list(range(N_CORES)),
                               trace=trace)
    if _sink is not None:
        _sink.append(res)
    return combine_outputs(res.results, b_qkv, w_out, b_out)



# revision 14
# speedup vs baseline: 19.9388x; 19.9388x over previous
"""Causal self-attention (B=4, T=2048, C=1024, H=16, D=64) on 8 TRN2 NeuronCores.

Sharding: batch x head-group. Core c handles batch b = c//2 and heads
hg*8..hg*8+8 where hg = c%2 (data parallel on batch, tensor parallel on heads;
w_qkv column-sharded, w_out row-sharded). Each core is fully independent; the
host sums the two per-batch partial outputs and adds the bias terms.

Precision strategy (rel-err budget 2e-2; fp8 error concentrates in the first
~64 query rows where softmax has no dilution, so rows 0-511 get a high-
precision path):
  - Q/K projection: fp32r matmuls (the path that feeds exp).
  - Q/K storage + S=QK^T: bf16 (adds ~3e-3 to S; harmless).
  - V projection, PV, out projection: fp8 e4m3 with DoubleRow perf mode
    (2 contraction k-tiles per instruction at 0.5 cycles/row = 4x fp32r).
    w1v/w2 are host-prescaled x16 so their ~N(0,1/32) entries stay out of
    e4m3 subnormals; the x256 net scale is divided out on the y evacuation.
  - P = exp(S/8 - 2.5): the global shift keeps P <= ~e^3.4 << 240 (e4m3 max)
    and cancels in the softmax normalization.
  - Query rows 0-511 (i-chunk 0): PV and out-projection run in bf16 off
    unquantized V0/OP0 copies.

Structure: one fused loop over (i-chunk, head-pair). Per iteration the PE
stream gets QKV-projection matmuls for this i-chunk's x chunk (dense filler
that hides exp latency) followed by S -> [mask matmuls] -> exp (ACT) -> PV
chains, then the out-projection for the finished i-chunk. Causal masking is
done by accumulating -1e9 onto masked S entries in PSUM with one extra
matmul per diagonal tile (host-constant indicator matrices), so exp produces
exact zeros and no post-exp masking pass is needed.

Normalization: PV's [V|1] ones column emits the softmax denominator row;
reciprocal (DVE) -> partition_broadcast (GPSIMD) -> multiply (DVE, writes
fp8/bf16 O^T directly from PSUM; the h2=1 half goes via a staging tile +
SBUF DMA because DVE cannot shift partition base).
"""

import numpy as np

import concourse.bass as bass
import concourse.bacc as bacc
import concourse.mybir as mybir
from concourse.tile import TileContext

# ---- problem constants (hardcoded per contract) ----
B, T, C = 4, 2048, 1024
H_GLOBAL, D = 16, 64
HL = 8                      # local heads per core
N_CORES = 8
P = 128
KT_C = C // P               # 8 contraction tiles over C
NT = T // P                 # 16 t-tiles
IB = 512                    # query block (i-chunk)
NIC = T // IB               # 4 i-chunks
XB = 512                    # x stream chunk width (t columns)
W8SCALE = 16.0              # host prescale for w1v / w2 in fp8
F32 = mybir.dt.float32
F32R = mybir.dt.float32r
BF16 = mybir.dt.bfloat16
FP8 = mybir.dt.float8e4
DR = mybir.MatmulPerfMode.DoubleRow
SCALE = 1.0 / np.sqrt(D).astype(np.float32)
ESHIFT = 2.5                # exp(S*SCALE - ESHIFT); cancels in softmax
NEG = -1.0e9


def round_f32r(a):
    """Host-side RNE rounding of fp32 to the fp32r bit format (11-bit
    explicit mantissa). Matches libwalrus fp32_to_fp32r exactly."""
    u = np.ascontiguousarray(a, dtype=np.float32).view(np.uint32).astype(np.uint64)
    low = u & 0xFFF
    keep = u >> 12
    roundup = (low > 0x800) | ((low == 0x800) & ((keep & 1) == 1))
    return ((keep + roundup.astype(np.uint64)) << 12).astype(np.uint32).view(
        np.float32)


def build_nc(repeat=1, phases="BCD"):
    nc = bacc.Bacc("TRN2", target_bir_lowering=False)
    EXPF = mybir.ActivationFunctionType.Exp

    xT = nc.dram_tensor("xT", [C, T], F32R, kind="ExternalInput").ap()
    xT8 = nc.dram_tensor("xT8", [C, T], FP8, kind="ExternalInput").ap()
    w1qk = nc.dram_tensor("w1qk", [C, 2 * HL * D], F32R,
                          kind="ExternalInput").ap()
    w1v8 = nc.dram_tensor("w1v8", [C, HL * D], FP8, kind="ExternalInput").ap()
    w1vr = nc.dram_tensor("w1vr", [C, HL * D], F32R, kind="ExternalInput").ap()
    b1 = nc.dram_tensor("b1", [2 * HL * D], F32, kind="ExternalInput").ap()
    w28 = nc.dram_tensor("w28", [HL * D, C], FP8, kind="ExternalInput").ap()
    w2r = nc.dram_tensor("w2r", [HL * D, C], BF16, kind="ExternalInput").ap()
    # [ -1e9*I | ones | strict-lower ]  (see make_masks)
    masks = nc.dram_tensor("masks", [P, 3 * P], BF16, kind="ExternalInput").ap()
    y = nc.dram_tensor("y", [T, C], F32, kind="ExternalOutput").ap()

    with TileContext(nc) as tc:
      for _rep in range(repeat):
        with tc.tile_pool(name="persist", bufs=1) as persist:
            b1_sb = persist.tile([P, 8], F32, tag="b1", name="b1_sb")
            nc.sync.dma_start(out=b1_sb[:, :], in_=b1.rearrange("(m p) -> p m", p=P))
            mk = persist.tile([P, 3 * P], BF16, tag="mk", name="mk")
            nc.sync.dma_start(out=mk[:, :], in_=masks)

            # startup DMAs in consumer order, two halves each so the first
            # matmuls start early without flooding the SP queue
            w1vr_sb = persist.tile([P, KT_C, HL * D], F32R, name="w1vr_sb")
            w1qk_sb = persist.tile([P, KT_C, 2 * HL * D], F32R, name="w1qk_sb")
            w1v8_sb = persist.tile([P, KT_C, HL * D], FP8, name="w1v8_sb")
            # w2 is first needed at the out-projection of i-chunk 0 (~60us
    # in); deferred into the loop so it doesn't compete with startup DMAs
            w28_sb = persist.tile([P, HL * D // P, C], FP8, name="w28_sb")
            w2r_sb = persist.tile([P, HL * D // P, C], BF16, name="w2r_sb")

            QT = [persist.tile([P, T], BF16, tag=f"QT{pr}", name=f"QT{pr}")
                  for pr in range(HL // 2)]
            KT = [persist.tile([P, T], BF16, tag=f"KT{pr}", name=f"KT{pr}")
                  for pr in range(HL // 2)]
            # fp8 V in t-tile pairs: [p, pair-slot, head*(D+1)]
            # per-head slot padded to D+2 so the DoubleRow weight AP's
            # pair stride (8*66 = 528 bytes) is 16-byte aligned
            V8 = [persist.tile([P, 2, HL * (D + 2)], FP8, tag=f"V8{tp}",
                               name=f"V8{tp}") for tp in range(NT // 2)]
            # bf16 V for j-tiles 0..3 (queries < 512)
            V0 = [persist.tile([P, HL * (D + 1)], BF16, tag=f"V0{tt}",
                               name=f"V0{tt}") for tt in range(4)]
            # fp8 O^T in head-pair groups: [p=(h2,d), pr-slot, t]
            OP8 = [persist.tile([P, 2, T], FP8, tag=f"OP8{i}", name=f"OP8{i}")
                   for i in range(2)]
            OP0 = [persist.tile([P, 2, IB], BF16, tag=f"OP0{i}", name=f"OP0{i}")
                   for i in range(2)]

            ones_f32 = persist.tile([P, HL], F32, tag="ones", name="ones_f32")
            nc.vector.memset(ones_f32[:, :], 1.0)
            ebias = persist.tile([P, 1], F32, tag="ebias", name="ebias")
            nc.vector.memset(ebias[:, :], -float(ESHIFT))
            for tp in range(NT // 2):
                for g in range(2):
                    nc.vector.tensor_copy(
                        out=V8[tp].rearrange("p g (h x) -> p g h x", x=D + 2)[
                            :, g, :, D:D + 1],
                        in_=ones_f32.rearrange("p (h o) -> p h o", o=1)[:, :, :])
            for tt in range(4):
                nc.vector.tensor_copy(
                    out=V0[tt].rearrange("p (h x) -> p h x", x=D + 1)[
                        :, :, D:D + 1],
                    in_=ones_f32.rearrange("p (h o) -> p h o", o=1)[:, :, :])

            with tc.tile_pool(name="xs", bufs=2) as xs, \
                 tc.tile_pool(name="pp", bufs=2, space="PSUM") as pp, \
                 tc.tile_pool(name="sp", bufs=2, space="PSUM") as spool, \
                 tc.tile_pool(name="op", bufs=1, space="PSUM") as opool, \
                 tc.tile_pool(name="pt", bufs=6) as ptp, \
                 tc.tile_pool(name="rc", bufs=2) as rcp, \
                 tc.tile_pool(name="bc", bufs=2) as bcp, \
                 tc.tile_pool(name="st", bufs=2) as stp, \
                 tc.tile_pool(name="yo", bufs=3) as yop:
                # ---- out projection for i-chunk dic's 4 t-tiles ----
                def emit_outproj(dic):
                    for ttl in range(IB // P):
                        tt = dic * (IB // P) + ttl
                        for cc in range(C // 512):
                            yp = pp.tile([P, 512], F32, tag="pp", name="yp")
                            if dic == 0:
                                for i in range(2):
                                    for g in range(2):
                                        nc.tensor.matmul(
                                            yp[:, :],
                                            lhsT=OP0[i][:, g,
                                                        ttl * P:(ttl + 1) * P],
                                            rhs=w2r_sb[:, 2 * i + g,
                                                       cc * 512:(cc + 1) * 512],
                                            start=(i == 0 and g == 0),
                                            stop=(i == 1 and g == 1))
                            else:
                                for i in range(2):
                                    nc.tensor.matmul(
                                        yp[:, :],
                                        lhsT=OP8[i][:, :, tt * P:(tt + 1) * P],
                                        rhs=w28_sb[:, 2 * i:2 * i + 2,
                                                   cc * 512:(cc + 1) * 512],
                                        start=(i == 0), stop=(i == 1),
                                        perf_mode=DR)
                            ysb = yop.tile([P, 512], F32, tag="ysb", name="ysb")
                            if dic == 0:
                                nc.vector.tensor_copy(out=ysb[:, :],
                                                      in_=yp[:, :])
                            else:
                                nc.vector.tensor_scalar_mul(
                                    ysb[:, :], yp[:, :],
                                    1.0 / (W8SCALE * W8SCALE))
                            nc.sync.dma_start(
                                out=y[tt * P:(tt + 1) * P,
                                      cc * 512:(cc + 1) * 512],
                                in_=ysb[:, :])

                xcs, xc8s = {}, {}
                pending = None

                def attn_pv(aic, apr, aops, pg, pts):
                    anjt = (aic + 1) * (IB // P)
                    jt0 = 2 * pg
                    rel0 = max(0, jt0 * P - aic * IB)
                    for h2 in range(2):
                        h = apr * 2 + h2
                        pt = pts[h2]
                        if aic == 0:
                            for jl, jt in enumerate((jt0, jt0 + 1)):
                                nc.tensor.matmul(
                                    aops[h2][:, rel0:IB],
                                    lhsT=V0[jt][:, h * (D + 1):
                                                (h + 1) * (D + 1)],
                                    rhs=pt[:, jl * IB + rel0:(jl + 1) * IB],
                                    start=(jt == 0),
                                    stop=(jt == anjt - 1))
                        else:
                            nc.tensor.matmul(
                                aops[h2][:, rel0:IB],
                                lhsT=V8[pg][:, :, h * (D + 2):
                                            h * (D + 2) + D + 1],
                                rhs=pt.rearrange("p (g i) -> p g i", g=2)[
                                    :, :, rel0:IB],
                                start=(pg == 0),
                                stop=(pg == anjt // 2 - 1),
                                perf_mode=DR)

                def attn_finish(aic, apr, aops, pg, pts):
                    attn_pv(aic, apr, aops, pg, pts)
                    # normalize: 1/denom broadcast, write O^T.  DVE cannot
                    # shift partition base: reciprocal lands at partition D;
                    # gpsimd broadcast moves it; h2=1 goes via staging + DMA.
                    for h2 in range(2):
                        rec = rcp.tile([D + 1, IB], F32, tag="rec", name="rec")
                        nc.vector.reciprocal(rec[D:D + 1, :],
                                             aops[h2][D:D + 1, :])
                        # move the recip row to true partition 0 first:
                        # gpsimd partition_broadcast reads tile partition 0
                        # on HW regardless of the AP base
                        rr = rcp.tile([1, IB], F32, tag="rr", name="rr")
                        nc.sync.dma_start(out=rr[0:1, :], in_=rec[D:D + 1, :])
                        bcst = bcp.tile([D, IB], F32, tag="bc", name="bc")
                        nc.gpsimd.partition_broadcast(
                            bcst[:, :], rr[0:1, :], channels=D)
                        if aic == 0:
                            odst, odt = OP0[apr // 2], BF16
                            osl = lambda lo, hi: odst[lo:hi, apr % 2, :]
                        else:
                            odst, odt = OP8[apr // 2], FP8
                            osl = lambda lo, hi: odst[lo:hi, apr % 2,
                                                      aic * IB:(aic + 1) * IB]
                        if h2 == 0:
                            nc.vector.tensor_mul(
                                out=osl(0, D),
                                in0=aops[h2][0:D, :], in1=bcst[:, :])
                        else:
                            stg = stp.tile([D, IB], odt, tag="stg", name="stg")
                            nc.vector.tensor_mul(
                                out=stg[:, :],
                                in0=aops[h2][0:D, :], in1=bcst[:, :])
                            nc.sync.dma_start(out=osl(D, 2 * D), in_=stg[:, :])

                def load_xc(j, halves=1):
                    t = xs.tile([P, KT_C, XB], F32R, tag="xc", name="xc")
                    xcs[j] = t
                    hk = KT_C // halves
                    for h in range(halves):
                        nc.sync.dma_start(
                            out=t[:, h * hk:(h + 1) * hk, :],
                            in_=xT.rearrange("(k p) t -> p k t", p=P)[
                                :, h * hk:(h + 1) * hk, j * XB:(j + 1) * XB])
                    if j > 0:
                        t8 = xs.tile([P, KT_C, XB], FP8, tag="xc8", name="xc8")
                        xc8s[j] = t8
                        nc.sync.dma_start(
                            out=t8[:, :, :],
                            in_=xT8.rearrange("(k p) t -> p k t", p=P)[
                                :, :, j * XB:(j + 1) * XB])

                # startup: interleave x / w1 halves in first-use order
                load_xc(0, halves=2)
                for h in range(2):
                    nc.sync.dma_start(
                        out=w1vr_sb[:, h * 4:(h + 1) * 4, :],
                        in_=w1vr.rearrange("(k p) m -> p k m", p=P)[
                            :, h * 4:(h + 1) * 4, :])
                for h in range(2):
                    nc.sync.dma_start(
                        out=w1qk_sb[:, h * 4:(h + 1) * 4, :],
                        in_=w1qk.rearrange("(k p) m -> p k m", p=P)[
                            :, h * 4:(h + 1) * 4, :])

                for ic in range(NIC):
                    xc = xcs.pop(ic)
                    xc8 = xc8s.pop(ic, None)

                    # ---- V projection for this chunk's 4 t-tiles ----
                    for ttl in range(XB // P):
                        tt = ic * (XB // P) + ttl
                        vp = pp.tile([P, HL * D], F32, tag="pp", name="vp")
                        if tt < 4:
                            for k in range(KT_C):
                                nc.tensor.matmul(
                                    vp[:, :],
                                    lhsT=xc[:, k, ttl * P:(ttl + 1) * P],
                                    rhs=w1vr_sb[:, k, :],
                                    start=(k == 0), stop=(k == KT_C - 1))
                            nc.vector.tensor_copy(
                                out=V0[tt].rearrange("p (h x) -> p h x",
                                                     x=D + 1)[:, :, 0:D],
                                in_=vp.rearrange("p (h x) -> p h x", x=D))
                            nc.vector.tensor_scalar_mul(
                                V8[tt // 2].rearrange(
                                    "p g (h x) -> p g h x", x=D + 2)[
                                    :, tt % 2, :, 0:D],
                                vp.rearrange("p (h x) -> p h x", x=D),
                                float(W8SCALE))
                        else:
                            for k in range(0, KT_C, 2):
                                nc.tensor.matmul(
                                    vp[:, :],
                                    lhsT=xc8[:, k:k + 2, ttl * P:(ttl + 1) * P],
                                    rhs=w1v8_sb[:, k:k + 2, :],
                                    start=(k == 0), stop=(k == KT_C - 2),
                                    perf_mode=DR)
                            nc.vector.tensor_copy(
                                out=V8[tt // 2].rearrange(
                                    "p g (h x) -> p g h x", x=D + 2)[
                                    :, tt % 2, :, 0:D],
                                in_=vp.rearrange("p (h x) -> p h x", x=D))

                    # prefetch next i-chunk's x while this one computes
                    if ic + 1 < NIC:
                        load_xc(ic + 1)
                    if ic == 0:
                        nc.sync.dma_start(
                            out=w1v8_sb[:, :, :],
                            in_=w1v8.rearrange("(k p) m -> p k m", p=P))
                        nc.sync.dma_start(
                            out=w28_sb[:, :, :],
                            in_=w28.rearrange("(k p) c -> p k c", p=P))
                        nc.sync.dma_start(
                            out=w2r_sb[:, :, :],
                            in_=w2r.rearrange("(k p) c -> p k c", p=P))
                    for pr in range(HL // 2):
                        # ---- Q/K projection piece for this (ic, pr) ----
                        for which, dst, boff in ((0, QT[pr], 0),
                                                 (1, KT[pr], 4)):
                            qp = pp.tile([P, XB], F32, tag="pp", name="qp")
                            for k in range(KT_C):
                                nc.tensor.matmul(
                                    qp[:, :],
                                    lhsT=w1qk_sb[:, k, which * HL * D + pr * P:
                                                 which * HL * D + (pr + 1) * P],
                                    rhs=xc[:, k, :],
                                    start=(k == 0), stop=(k == KT_C - 1))
                            nc.vector.tensor_scalar_add(
                                dst[:, ic * XB:(ic + 1) * XB], qp[:, :],
                                b1_sb[:, boff + pr:boff + pr + 1])

                        # flush the previous head-pair's attention tail now
                        # that the projections above cover its exp latency
                        if pending is not None:
                            attn_finish(*pending)
                            pending = None
                        if pr == 0 and ic > 0:
                            emit_outproj(ic - 1)

                        # ---- attention piece (queries ic*512..+512) ----
                        njt = (ic + 1) * (IB // P)
                        ops = [opool.tile([D + 1, IB], F32, tag=f"o{h2}",
                                          name=f"o{h2}") for h2 in range(2)]
                        prev = None
                        for pg in range(njt // 2):
                            jt0 = 2 * pg
                            rel0 = max(0, jt0 * P - ic * IB)
                            pts = []
                            for h2 in range(2):
                                hs = h2 * D
                                sp = spool.tile([P, 2 * IB], F32, tag="sp",
                                                name="sp")
                                for jl, jt in enumerate((jt0, jt0 + 1)):
                                    ondiag = jt * P >= ic * IB
                                    nc.tensor.matmul(
                                        sp[:, jl * IB + rel0:(jl + 1) * IB],
                                        lhsT=KT[pr][hs:hs + D,
                                                    jt * P:(jt + 1) * P],
                                        rhs=QT[pr][hs:hs + D,
                                                   ic * IB + rel0:(ic + 1) * IB],
                                        start=True, stop=not ondiag)
                                    if ondiag:
                                        if jl == 0:
                                            # aligned triangle: strict lower
                                            nc.tensor.matmul(
                                                sp[:, rel0:rel0 + P],
                                                lhsT=mk[:, 0:P],
                                                rhs=mk[:, 2 * P:3 * P],
                                                start=False, stop=True)
                                        else:
                                            # full strip + triangle (256 wide)
                                            nc.tensor.matmul(
                                                sp[:, IB + rel0:IB + rel0 + 2 * P],
                                                lhsT=mk[:, 0:P],
                                                rhs=mk[:, P:3 * P],
                                                start=False, stop=True)
                                pt = ptp.tile([P, 2 * IB],
                                              BF16 if ic == 0 else FP8,
                                              tag="pt0" if ic == 0 else "pt8",
                                              name="pt")
                                pts.append(pt)
                                if rel0 == 0:
                                    nc.scalar.activation(
                                        pt[:, 0:2 * IB], sp[:, 0:2 * IB],
                                        EXPF, bias=ebias[:, 0:1],
                                        scale=float(SCALE))
                                else:
                                    for jl in range(2):
                                        nc.scalar.activation(
                                            pt[:, jl * IB + rel0:(jl + 1) * IB],
                                            sp[:, jl * IB + rel0:(jl + 1) * IB],
                                            EXPF, bias=ebias[:, 0:1],
                                            scale=float(SCALE))
                            if prev is not None:
                                attn_pv(ic, pr, ops, *prev)
                            prev = (pg, pts)
                        pending = (ic, pr, ops) + prev
                if pending is not None:
                    attn_finish(*pending)
                emit_outproj(NIC - 1)

    nc.compile()
    return nc


_NC_CACHE = None


def _get_nc():
    global _NC_CACHE
    if _NC_CACHE is None:
        _NC_CACHE = build_nc()
    return _NC_CACHE


def make_masks():
    """[ -1e9*I | ones | strict-lower(k>i) ] mask-matmul constants.
    Aligned diagonal tile: lhsT = -1e9*I, rhs = strict-lower slice ->
    adds -1e9 where j > i. Second tile of a diagonal pair: rhs spans
    [ones | strict-lower] (256 cols): first 128 columns fully masked."""
    ident = np.eye(P, dtype=np.float32) * NEG
    ones = np.ones((P, P), dtype=np.float32)
    ua = np.tril(np.ones((P, P), dtype=np.float32), -1)
    return np.concatenate([ident, ones, ua], axis=1)


def shard_inputs(x, w_qkv, b_qkv, w_out):
    """Build the 8 per-core input maps."""
    import ml_dtypes
    x = np.asarray(x, dtype=np.float32)
    w_qkv = np.asarray(w_qkv, dtype=np.float32)
    b_qkv = np.asarray(b_qkv, dtype=np.float32)
    w_out = np.asarray(w_out, dtype=np.float32)

    def q8(a):
        return np.ascontiguousarray(a).astype(ml_dtypes.float8_e4m3)

    masks = make_masks().astype(ml_dtypes.bfloat16)
    in_maps = []
    for core in range(N_CORES):
        b, hg = core // 2, core % 2
        cs = hg * HL * D              # 512-wide contiguous head-group slice
        w1qk_c = np.ascontiguousarray(np.concatenate(
            [w_qkv[:, cs:cs + HL * D],
             w_qkv[:, C + cs:C + cs + HL * D]], axis=1))
        w1v_c = np.ascontiguousarray(w_qkv[:, 2 * C + cs:2 * C + cs + HL * D])
        b1_c = np.ascontiguousarray(np.concatenate(
            [b_qkv[cs:cs + HL * D], b_qkv[C + cs:C + cs + HL * D]]))
        w2_c = np.ascontiguousarray(w_out[cs:cs + HL * D, :])
        in_maps.append({
            "xT": round_f32r(x[b].T),
            "xT8": q8(x[b].T),
            "w1qk": round_f32r(w1qk_c),
            "w1v8": q8(w1v_c * W8SCALE),
            "w1vr": round_f32r(w1v_c),
            "b1": b1_c,
            "w28": q8(w2_c * W8SCALE),
            "w2r": w2_c.astype(ml_dtypes.bfloat16),
            "masks": masks,
        })
    return in_maps


def combine_outputs(results, b_qkv, w_out, b_out):
    """Sum per-batch partials from the two head-group cores + bias terms."""
    bias_vec = (np.asarray(b_qkv[2 * C:3 * C], dtype=np.float32) @
                np.asarray(w_out, dtype=np.float32) +
                np.asarray(b_out, dtype=np.float32))
    y = np.empty((B, T, C), dtype=np.float32)
    for b in range(B):
        y[b] = results[2 * b]["y"] + results[2 * b + 1]["y"] + bias_vec
    return y


def kernel(x, w_qkv, b_qkv, w_out, b_out, *, trace=False, _sink=None):
    from concourse.bass_utils import run_bass_kernel_spmd
    nc = _get_nc()
    in_maps = shard_inputs(x, w_qkv, b_qkv, w_out)
    res = run_bass_kernel_spmd(nc, in_maps, core_ids=list(range(N_CORES)),
                               trace=trace)
    if _sink is not None:
        _sink.append(res)
    return combine_outputs(res.results, b_qkv, w_out, b_out)


# revision 15
# speedup vs baseline: 42.8317x; 2.1482x over previous
"""Causal self-attention (B=4, T=2048, C=1024, H=16, D=64) on 8 TRN2 NeuronCores.

Sharding: batch x head-group. Core c handles batch b = c//2 and heads
hg*8..hg*8+8 where hg = c%2 (data parallel on batch, tensor parallel on heads;
w_qkv column-sharded, w_out row-sharded). Each core is fully independent; the
host sums the two per-batch partial outputs and adds the bias terms.

Precision strategy (rel-err budget 2e-2; fp8 error concentrates in the first
~64 query rows where softmax has no dilution, so rows 0-511 get a high-
precision path):
  - Q/K projection: fp32r matmuls (the path that feeds exp).
  - Q/K storage + S=QK^T: bf16 (adds ~3e-3 to S; harmless).
  - V projection, PV, out projection: fp8 e4m3 with DoubleRow perf mode
    (2 contraction k-tiles per instruction at 0.5 cycles/row = 4x fp32r).
    w1v/w2 are host-prescaled x16 so their ~N(0,1/32) entries stay out of
    e4m3 subnormals; the x256 net scale is divided out on the y evacuation.
  - P = exp(S/8 - 2.5): the global shift keeps P <= ~e^3.4 << 240 (e4m3 max)
    and cancels in the softmax normalization.
  - Query rows 0-511 (i-chunk 0): PV and out-projection run in bf16 off
    unquantized V0/OP0 copies.

Structure: one fused loop over (i-chunk, head-pair). Per iteration the PE
stream gets QKV-projection matmuls for this i-chunk's x chunk (dense filler
that hides exp latency) followed by S -> [mask matmuls] -> exp (ACT) -> PV
chains, then the out-projection for the finished i-chunk. Causal masking is
done by accumulating -1e9 onto masked S entries in PSUM with one extra
matmul per diagonal tile (host-constant indicator matrices), so exp produces
exact zeros and no post-exp masking pass is needed.

Normalization: PV's [V|1] ones column emits the softmax denominator row;
reciprocal (DVE) -> partition_broadcast (GPSIMD) -> multiply (DVE, writes
fp8/bf16 O^T directly from PSUM; the h2=1 half goes via a staging tile +
SBUF DMA because DVE cannot shift partition base).
"""

import numpy as np

import concourse.bass as bass
import concourse.bacc as bacc
import concourse.mybir as mybir
from concourse.tile import TileContext

# ---- problem constants (hardcoded per contract) ----
B, T, C = 4, 2048, 1024
H_GLOBAL, D = 16, 64
HL = 8                      # local heads per core
N_CORES = 8
P = 128
KT_C = C // P               # 8 contraction tiles over C
NT = T // P                 # 16 t-tiles
IB = 512                    # query block (i-chunk)
NIC = T // IB               # 4 i-chunks
XB = 512                    # x stream chunk width (t columns)
W8SCALE = 16.0              # host prescale for w1v / w2 in fp8
F32 = mybir.dt.float32
F32R = mybir.dt.float32r
BF16 = mybir.dt.bfloat16
FP8 = mybir.dt.float8e4
DR = mybir.MatmulPerfMode.DoubleRow
SCALE = 1.0 / np.sqrt(D).astype(np.float32)
ESHIFT = 2.5                # exp(S*SCALE - ESHIFT); cancels in softmax
QKSCALE = 16.0              # host prescale on w1qk (both paths)
NEG = -1.0e9


def round_f32r(a):
    """Host-side RNE rounding of fp32 to the fp32r bit format (11-bit
    explicit mantissa). Matches libwalrus fp32_to_fp32r exactly."""
    u = np.ascontiguousarray(a, dtype=np.float32).view(np.uint32).astype(np.uint64)
    low = u & 0xFFF
    keep = u >> 12
    roundup = (low > 0x800) | ((low == 0x800) & ((keep & 1) == 1))
    return ((keep + roundup.astype(np.uint64)) << 12).astype(np.uint32).view(
        np.float32)


def build_nc(repeat=1, phases="BCD"):
    nc = bacc.Bacc("TRN2", target_bir_lowering=False)
    EXPF = mybir.ActivationFunctionType.Exp

    xT = nc.dram_tensor("xT", [C, T], F32R, kind="ExternalInput").ap()
    xT8 = nc.dram_tensor("xT8", [C, T], FP8, kind="ExternalInput").ap()
    w1qk = nc.dram_tensor("w1qk", [C, 2 * HL * D], F32R,
                          kind="ExternalInput").ap()
    w1qk8 = nc.dram_tensor("w1qk8", [C, 2 * HL * D], FP8,
                           kind="ExternalInput").ap()
    w1v8 = nc.dram_tensor("w1v8", [C, HL * D], FP8, kind="ExternalInput").ap()
    w1vr = nc.dram_tensor("w1vr", [C, HL * D], F32R, kind="ExternalInput").ap()
    b1 = nc.dram_tensor("b1", [2 * HL * D], F32, kind="ExternalInput").ap()
    w28 = nc.dram_tensor("w28", [HL * D, C], FP8, kind="ExternalInput").ap()
    w2r = nc.dram_tensor("w2r", [HL * D, C], BF16, kind="ExternalInput").ap()
    # [ -1e9*I | ones | strict-lower ]  (see make_masks)
    masks = nc.dram_tensor("masks", [P, 3 * P], BF16, kind="ExternalInput").ap()
    y = nc.dram_tensor("y", [T, C], F32, kind="ExternalOutput").ap()

    with TileContext(nc) as tc:
      for _rep in range(repeat):
        with tc.tile_pool(name="persist", bufs=1) as persist:
            b1_sb = persist.tile([P, 8], F32, tag="b1", name="b1_sb")
            nc.sync.dma_start(out=b1_sb[:, :], in_=b1.rearrange("(m p) -> p m", p=P))
            mk = persist.tile([P, 3 * P], BF16, tag="mk", name="mk")
            nc.sync.dma_start(out=mk[:, :], in_=masks)

            # startup DMAs in consumer order, two halves each so the first
            # matmuls start early without flooding the SP queue
            w1vr_sb = persist.tile([P, KT_C, HL * D], F32R, name="w1vr_sb")
            w1qk_sb = persist.tile([P, KT_C, 2 * HL * D], F32R, name="w1qk_sb")
            w1v8_sb = persist.tile([P, KT_C, HL * D], FP8, name="w1v8_sb")
            w1qk8_sb = persist.tile([P, KT_C, 2 * HL * D], FP8,
                                    name="w1qk8_sb")
            # w2 is first needed at the out-projection of i-chunk 0 (~60us
    # in); deferred into the loop so it doesn't compete with startup DMAs
            w28_sb = persist.tile([P, HL * D // P, C], FP8, name="w28_sb")
            w2r_sb = persist.tile([P, HL * D // P, C], BF16, name="w2r_sb")

            QT = [persist.tile([P, T], BF16, tag=f"QT{pr}", name=f"QT{pr}")
                  for pr in range(HL // 2)]
            KT = [persist.tile([P, T], BF16, tag=f"KT{pr}", name=f"KT{pr}")
                  for pr in range(HL // 2)]
            # fp8 V in t-tile pairs: [p, pair-slot, head*(D+1)]
            # per-head slot padded to D+2 so the DoubleRow weight AP's
            # pair stride (8*66 = 528 bytes) is 16-byte aligned
            V8 = [persist.tile([P, 2, HL * (D + 2)], FP8, tag=f"V8{tp}",
                               name=f"V8{tp}") for tp in range(NT // 2)]
            # bf16 V for j-tiles 0..3 (queries < 512)
            V0 = [persist.tile([P, HL * (D + 1)], BF16, tag=f"V0{tt}",
                               name=f"V0{tt}") for tt in range(4)]
            # fp8 O^T in head-pair groups: [p=(h2,d), pr-slot, t]
            OP8 = [persist.tile([P, 2, T], FP8, tag=f"OP8{i}", name=f"OP8{i}")
                   for i in range(2)]
            OP0 = [persist.tile([P, 2, IB], BF16, tag=f"OP0{i}", name=f"OP0{i}")
                   for i in range(2)]

            ones_f32 = persist.tile([P, HL], F32, tag="ones", name="ones_f32")
            nc.vector.memset(ones_f32[:, :], 1.0)
            ebias = persist.tile([P, 1], F32, tag="ebias", name="ebias")
            nc.vector.memset(ebias[:, :], -float(ESHIFT))
            for tp in range(NT // 2):
                for g in range(2):
                    nc.vector.tensor_copy(
                        out=V8[tp].rearrange("p g (h x) -> p g h x", x=D + 2)[
                            :, g, :, D:D + 1],
                        in_=ones_f32.rearrange("p (h o) -> p h o", o=1)[:, :, :])
            for tt in range(4):
                nc.vector.tensor_copy(
                    out=V0[tt].rearrange("p (h x) -> p h x", x=D + 1)[
                        :, :, D:D + 1],
                    in_=ones_f32.rearrange("p (h o) -> p h o", o=1)[:, :, :])

            with tc.tile_pool(name="xs", bufs=2) as xs, \
                 tc.tile_pool(name="pp", bufs=2, space="PSUM") as pp, \
                 tc.tile_pool(name="sp", bufs=2, space="PSUM") as spool, \
                 tc.tile_pool(name="op", bufs=1, space="PSUM") as opool, \
                 tc.tile_pool(name="pt", bufs=6) as ptp, \
                 tc.tile_pool(name="rc", bufs=2) as rcp, \
                 tc.tile_pool(name="bc", bufs=2) as bcp, \
                 tc.tile_pool(name="st", bufs=2) as stp, \
                 tc.tile_pool(name="yo", bufs=3) as yop:
                # ---- out projection for i-chunk dic's 4 t-tiles ----
                def emit_outproj(dic):
                    for ttl in range(IB // P):
                        tt = dic * (IB // P) + ttl
                        for cc in range(C // 512):
                            yp = pp.tile([P, 512], F32, tag="pp", name="yp")
                            if dic == 0:
                                for i in range(2):
                                    for g in range(2):
                                        nc.tensor.matmul(
                                            yp[:, :],
                                            lhsT=OP0[i][:, g,
                                                        ttl * P:(ttl + 1) * P],
                                            rhs=w2r_sb[:, 2 * i + g,
                                                       cc * 512:(cc + 1) * 512],
                                            start=(i == 0 and g == 0),
                                            stop=(i == 1 and g == 1))
                            else:
                                for i in range(2):
                                    nc.tensor.matmul(
                                        yp[:, :],
                                        lhsT=OP8[i][:, :, tt * P:(tt + 1) * P],
                                        rhs=w28_sb[:, 2 * i:2 * i + 2,
                                                   cc * 512:(cc + 1) * 512],
                                        start=(i == 0), stop=(i == 1),
                                        perf_mode=DR)
                            ysb = yop.tile([P, 512], F32, tag="ysb", name="ysb")
                            if dic == 0:
                                nc.vector.tensor_copy(out=ysb[:, :],
                                                      in_=yp[:, :])
                            else:
                                nc.vector.tensor_scalar_mul(
                                    ysb[:, :], yp[:, :],
                                    1.0 / (W8SCALE * W8SCALE))
                            nc.sync.dma_start(
                                out=y[tt * P:(tt + 1) * P,
                                      cc * 512:(cc + 1) * 512],
                                in_=ysb[:, :])

                xcs, xc8s = {}, {}
                pending = None

                def attn_pv(aic, apr, aops, pg, pts):
                    anjt = (aic + 1) * (IB // P)
                    jt0 = 2 * pg
                    rel0 = max(0, jt0 * P - aic * IB)
                    for h2 in range(2):
                        h = apr * 2 + h2
                        pt = pts[h2]
                        if aic == 0:
                            for jl, jt in enumerate((jt0, jt0 + 1)):
                                nc.tensor.matmul(
                                    aops[h2][:, rel0:IB],
                                    lhsT=V0[jt][:, h * (D + 1):
                                                (h + 1) * (D + 1)],
                                    rhs=pt[:, jl * IB + rel0:(jl + 1) * IB],
                                    start=(jt == 0),
                                    stop=(jt == anjt - 1))
                        else:
                            nc.tensor.matmul(
                                aops[h2][:, rel0:IB],
                                lhsT=V8[pg][:, :, h * (D + 2):
                                            h * (D + 2) + D + 1],
                                rhs=pt.rearrange("p (g i) -> p g i", g=2)[
                                    :, :, rel0:IB],
                                start=(pg == 0),
                                stop=(pg == anjt // 2 - 1),
                                perf_mode=DR)

                def attn_finish(aic, apr, aops, pg, pts):
                    attn_pv(aic, apr, aops, pg, pts)
                    # normalize: 1/denom broadcast, write O^T.  DVE cannot
                    # shift partition base: reciprocal lands at partition D;
                    # gpsimd broadcast moves it; h2=1 goes via staging + DMA.
                    for h2 in range(2):
                        rec = rcp.tile([D + 1, IB], F32, tag="rec", name="rec")
                        nc.vector.reciprocal(rec[D:D + 1, :],
                                             aops[h2][D:D + 1, :])
                        # move the recip row to true partition 0 first:
                        # gpsimd partition_broadcast reads tile partition 0
                        # on HW regardless of the AP base
                        rr = rcp.tile([1, IB], F32, tag="rr", name="rr")
                        nc.sync.dma_start(out=rr[0:1, :], in_=rec[D:D + 1, :])
                        bcst = bcp.tile([D, IB], F32, tag="bc", name="bc")
                        nc.gpsimd.partition_broadcast(
                            bcst[:, :], rr[0:1, :], channels=D)
                        if aic == 0:
                            odst, odt = OP0[apr // 2], BF16
                            osl = lambda lo, hi: odst[lo:hi, apr % 2, :]
                        else:
                            odst, odt = OP8[apr // 2], FP8
                            osl = lambda lo, hi: odst[lo:hi, apr % 2,
                                                      aic * IB:(aic + 1) * IB]
                        if h2 == 0:
                            nc.vector.tensor_mul(
                                out=osl(0, D),
                                in0=aops[h2][0:D, :], in1=bcst[:, :])
                        else:
                            stg = stp.tile([D, IB], odt, tag="stg", name="stg")
                            nc.vector.tensor_mul(
                                out=stg[:, :],
                                in0=aops[h2][0:D, :], in1=bcst[:, :])
                            nc.sync.dma_start(out=osl(D, 2 * D), in_=stg[:, :])

                def load_xc(j, halves=1):
                    t = xs.tile([P, KT_C, XB], F32R, tag="xc", name="xc")
                    xcs[j] = t
                    hk = KT_C // halves
                    for h in range(halves):
                        nc.sync.dma_start(
                            out=t[:, h * hk:(h + 1) * hk, :],
                            in_=xT.rearrange("(k p) t -> p k t", p=P)[
                                :, h * hk:(h + 1) * hk, j * XB:(j + 1) * XB])
                    if j > 0:
                        t8 = xs.tile([P, KT_C, XB], FP8, tag="xc8", name="xc8")
                        xc8s[j] = t8
                        nc.sync.dma_start(
                            out=t8[:, :, :],
                            in_=xT8.rearrange("(k p) t -> p k t", p=P)[
                                :, :, j * XB:(j + 1) * XB])

                # startup: interleave x / w1 halves in first-use order
                load_xc(0, halves=2)
                for h in range(2):
                    nc.sync.dma_start(
                        out=w1vr_sb[:, h * 4:(h + 1) * 4, :],
                        in_=w1vr.rearrange("(k p) m -> p k m", p=P)[
                            :, h * 4:(h + 1) * 4, :])
                for h in range(2):
                    nc.sync.dma_start(
                        out=w1qk_sb[:, h * 4:(h + 1) * 4, :],
                        in_=w1qk.rearrange("(k p) m -> p k m", p=P)[
                            :, h * 4:(h + 1) * 4, :])

                for ic in range(NIC):
                    xc = xcs.pop(ic)
                    xc8 = xc8s.pop(ic, None)

                    # ---- V projection for this chunk's 4 t-tiles ----
                    for ttl in range(XB // P):
                        tt = ic * (XB // P) + ttl
                        vp = pp.tile([P, HL * D], F32, tag="pp", name="vp")
                        if tt < 4:
                            for k in range(KT_C):
                                nc.tensor.matmul(
                                    vp[:, :],
                                    lhsT=xc[:, k, ttl * P:(ttl + 1) * P],
                                    rhs=w1vr_sb[:, k, :],
                                    start=(k == 0), stop=(k == KT_C - 1))
                            nc.vector.tensor_copy(
                                out=V0[tt].rearrange("p (h x) -> p h x",
                                                     x=D + 1)[:, :, 0:D],
                                in_=vp.rearrange("p (h x) -> p h x", x=D))
                            nc.vector.tensor_scalar_mul(
                                V8[tt // 2].rearrange(
                                    "p g (h x) -> p g h x", x=D + 2)[
                                    :, tt % 2, :, 0:D],
                                vp.rearrange("p (h x) -> p h x", x=D),
                                float(W8SCALE))
                        else:
                            for k in range(0, KT_C, 2):
                                nc.tensor.matmul(
                                    vp[:, :],
                                    lhsT=xc8[:, k:k + 2, ttl * P:(ttl + 1) * P],
                                    rhs=w1v8_sb[:, k:k + 2, :],
                                    start=(k == 0), stop=(k == KT_C - 2),
                                    perf_mode=DR)
                            nc.vector.tensor_copy(
                                out=V8[tt // 2].rearrange(
                                    "p g (h x) -> p g h x", x=D + 2)[
                                    :, tt % 2, :, 0:D],
                                in_=vp.rearrange("p (h x) -> p h x", x=D))

                    # prefetch next i-chunk's x while this one computes
                    if ic + 1 < NIC:
                        load_xc(ic + 1)
                    if ic == 0:
                        nc.sync.dma_start(
                            out=w1v8_sb[:, :, :],
                            in_=w1v8.rearrange("(k p) m -> p k m", p=P))
                        nc.sync.dma_start(
                            out=w1qk8_sb[:, :, :],
                            in_=w1qk8.rearrange("(k p) m -> p k m", p=P))
                        nc.sync.dma_start(
                            out=w28_sb[:, :, :],
                            in_=w28.rearrange("(k p) c -> p k c", p=P))
                        nc.sync.dma_start(
                            out=w2r_sb[:, :, :],
                            in_=w2r.rearrange("(k p) c -> p k c", p=P))
                    for pr in range(HL // 2):
                        # ---- Q/K projection piece for this (ic, pr) ----
                        for which, dst, boff in ((0, QT[pr], 0),
                                                 (1, KT[pr], 4)):
                            qp = pp.tile([P, XB], F32, tag="pp", name="qp")
                            col = which * HL * D + pr * P
                            if ic == 0:
                                # rows < 512: full-precision path
                                for k in range(KT_C):
                                    nc.tensor.matmul(
                                        qp[:, :],
                                        lhsT=w1qk_sb[:, k, col:col + P],
                                        rhs=xc[:, k, :],
                                        start=(k == 0), stop=(k == KT_C - 1))
                            else:
                                for k in range(0, KT_C, 2):
                                    nc.tensor.matmul(
                                        qp[:, :],
                                        lhsT=w1qk8_sb[:, k:k + 2, col:col + P],
                                        rhs=xc8[:, k:k + 2, :],
                                        start=(k == 0), stop=(k == KT_C - 2),
                                        perf_mode=DR)
                            nc.vector.tensor_scalar_add(
                                dst[:, ic * XB:(ic + 1) * XB], qp[:, :],
                                b1_sb[:, boff + pr:boff + pr + 1])

                        # flush the previous head-pair's attention tail now
                        # that the projections above cover its exp latency
                        if pending is not None:
                            attn_finish(*pending)
                            pending = None
                        if pr == 0 and ic > 0:
                            emit_outproj(ic - 1)

                        # ---- attention piece (queries ic*512..+512) ----
                        njt = (ic + 1) * (IB // P)
                        ops = [opool.tile([D + 1, IB], F32, tag=f"o{h2}",
                                          name=f"o{h2}") for h2 in range(2)]
                        prev = None
                        for pg in range(njt // 2):
                            jt0 = 2 * pg
                            rel0 = max(0, jt0 * P - ic * IB)
                            pts = []
                            for h2 in range(2):
                                hs = h2 * D
                                sp = spool.tile([P, 2 * IB], F32, tag="sp",
                                                name="sp")
                                for jl, jt in enumerate((jt0, jt0 + 1)):
                                    ondiag = jt * P >= ic * IB
                                    nc.tensor.matmul(
                                        sp[:, jl * IB + rel0:(jl + 1) * IB],
                                        lhsT=KT[pr][hs:hs + D,
                                                    jt * P:(jt + 1) * P],
                                        rhs=QT[pr][hs:hs + D,
                                                   ic * IB + rel0:(ic + 1) * IB],
                                        start=True, stop=not ondiag)
                                    if ondiag:
                                        if jl == 0:
                                            # aligned triangle: strict lower
                                            nc.tensor.matmul(
                                                sp[:, rel0:rel0 + P],
                                                lhsT=mk[:, 0:P],
                                                rhs=mk[:, 2 * P:3 * P],
                                                start=False, stop=True)
                                        else:
                                            # full strip + triangle (256 wide)
                                            nc.tensor.matmul(
                                                sp[:, IB + rel0:IB + rel0 + 2 * P],
                                                lhsT=mk[:, 0:P],
                                                rhs=mk[:, P:3 * P],
                                                start=False, stop=True)
                                pt = ptp.tile([P, 2 * IB],
                                              BF16 if ic == 0 else FP8,
                                              tag="pt0" if ic == 0 else "pt8",
                                              name="pt")
                                pts.append(pt)
                                esc = float(SCALE) / (QKSCALE * QKSCALE)
                                if rel0 == 0:
                                    nc.scalar.activation(
                                        pt[:, 0:2 * IB], sp[:, 0:2 * IB],
                                        EXPF, bias=ebias[:, 0:1], scale=esc)
                                else:
                                    for jl in range(2):
                                        nc.scalar.activation(
                                            pt[:, jl * IB + rel0:(jl + 1) * IB],
                                            sp[:, jl * IB + rel0:(jl + 1) * IB],
                                            EXPF, bias=ebias[:, 0:1],
                                            scale=esc)
                            if prev is not None:
                                attn_pv(ic, pr, ops, *prev)
                            prev = (pg, pts)
                        pending = (ic, pr, ops) + prev
                if pending is not None:
                    attn_finish(*pending)
                emit_outproj(NIC - 1)

    nc.compile()
    return nc


_NC_CACHE = None


def _get_nc():
    global _NC_CACHE
    if _NC_CACHE is None:
        _NC_CACHE = build_nc()
    return _NC_CACHE


def make_masks():
    """[ -1e9*I | ones | strict-lower(k>i) ] mask-matmul constants.
    Aligned diagonal tile: lhsT = -1e9*I, rhs = strict-lower slice ->
    adds -1e9 where j > i. Second tile of a diagonal pair: rhs spans
    [ones | strict-lower] (256 cols): first 128 columns fully masked."""
    ident = np.eye(P, dtype=np.float32) * NEG
    ones = np.ones((P, P), dtype=np.float32)
    ua = np.tril(np.ones((P, P), dtype=np.float32), -1)
    return np.concatenate([ident, ones, ua], axis=1)


def shard_inputs(x, w_qkv, b_qkv, w_out):
    """Build the 8 per-core input maps."""
    import ml_dtypes
    x = np.asarray(x, dtype=np.float32)
    w_qkv = np.asarray(w_qkv, dtype=np.float32)
    b_qkv = np.asarray(b_qkv, dtype=np.float32)
    w_out = np.asarray(w_out, dtype=np.float32)

    def q8(a):
        return np.ascontiguousarray(a).astype(ml_dtypes.float8_e4m3)

    masks = make_masks().astype(ml_dtypes.bfloat16)
    in_maps = []
    for core in range(N_CORES):
        b, hg = core // 2, core % 2
        cs = hg * HL * D              # 512-wide contiguous head-group slice
        w1qk_c = np.ascontiguousarray(np.concatenate(
            [w_qkv[:, cs:cs + HL * D],
             w_qkv[:, C + cs:C + cs + HL * D]], axis=1))
        w1v_c = np.ascontiguousarray(w_qkv[:, 2 * C + cs:2 * C + cs + HL * D])
        b1_c = np.ascontiguousarray(np.concatenate(
            [b_qkv[cs:cs + HL * D], b_qkv[C + cs:C + cs + HL * D]]))
        w2_c = np.ascontiguousarray(w_out[cs:cs + HL * D, :])
        in_maps.append({
            "xT": round_f32r(x[b].T),
            "xT8": q8(x[b].T),
            "w1qk": round_f32r(w1qk_c * W8SCALE),
            "w1qk8": q8(w1qk_c * W8SCALE),
            "w1v8": q8(w1v_c * W8SCALE),
            "w1vr": round_f32r(w1v_c),
            "b1": b1_c * W8SCALE,
            "w28": q8(w2_c * W8SCALE),
            "w2r": w2_c.astype(ml_dtypes.bfloat16),
            "masks": masks,
        })
    return in_maps


def combine_outputs(results, b_qkv, w_out, b_out):
    """Sum per-batch partials from the two head-group cores + bias terms."""
    bias_vec = (np.asarray(b_qkv[2 * C:3 * C], dtype=np.float32) @
                np.asarray(w_out, dtype=np.float32) +
                np.asarray(b_out, dtype=np.float32))
    y = np.empty((B, T, C), dtype=np.float32)
    for b in range(B):
        y[b] = results[2 * b]["y"] + results[2 * b + 1]["y"] + bias_vec
    return y


def kernel(x, w_qkv, b_qkv, w_out, b_out, *, trace=False, _sink=None):
    from concourse.bass_utils import run_bass_kernel_spmd
    nc = _get_nc()
    in_maps = shard_inputs(x, w_qkv, b_qkv, w_out)
    res = run_bass_kernel_spmd(nc, in_maps, core_ids=list(range(N_CORES)),
                               trace=trace)
    if _sink is not None:
        _sink.append(res)
    return combine_outputs(res.results, b_qkv, w_out, b_out)
